# revision 35
# baseline (speedup 1.0000x reference)
"""Multi-head attention Trainium2 Bass kernel, 8-way SPMD, v2.

Problem: nn_MultiHeadAttention (B=2, S=4096, D=512, H=8, Dk=64), fp32 I/O.

Sharding v2: (batch, head-pair). Core c = (batch c//4, head-pair c%4).
Each core computes its two heads' Q/K/V projections over the FULL 4096-row
sequence of its batch (no projection duplication), runs attention for those
heads, and produces a PARTIAL output projection (its 128 attended dims
through the matching w_o rows). The host sums the 4 partials per batch and
adds b_o.

Engine budget per core (the kernel is ACT-bound; HW-measured rates):
  ACT  : 256 exp instructions over [128,1024] PSUM scores, 1015 ns each
         (HW-measured stream rate) ~260 us  <- hard floor
  PE   : scores (head-pair row-group concurrency, 255 ns/pair measured)
         + AV (ones-column denom, 552 ns/pair) + projections  ~220 us
  DVE  : bias adds, normalize, psum->sbuf out copies           ~82 us
  DMA  : bf16 x/w loaded transposed via XBAR dma_start_transpose
         ([512,512] chunk call = 2.45 us measured; all on the SP hwdge
         queue - ACT-queue DMA dispatch races on this runtime), ~60 us

Scheduling rules this kernel is built around (all HW-verified here):
  - PE executes strictly in order: any matmul that waits on an ACT/DVE
    result blocks every later matmul. AV(kt) is therefore emitted after
    scores(kt+1), and the per-qc epilogue (normalize + out proj) is
    deferred into the next qc's kt loop in small PE batches.
  - Production (K/V/Q projections) is paced one chunk ahead of its
    consumption so its PSUM-wait never parks the PE queue.
  - All XBAR transposes are emitted upfront, ordered by need time; the
    SP DMA queue drains them (~2.5 us each) under the attention stream.

Key layout facts:
  - dma_start_transpose([128,512] DRAM bf16) -> [128,4,128] SBUF gives
    xT[p, dic, j] = x[j, dic*128+p] directly (verified on HW).
  - KT/QT [128, 4096] bf16 hold the head pair stacked (head0 dims on
    partitions 0..63, head1 on 64..127), so the two score matmuls use PE
    row-groups 0/64 and run concurrently (tile_position auto-derived).
  - V' = [V_h | 1] per head (65 cols): the AV matmul accumulates the
    softmax denominator in PSUM row 64 for free.

repeat/timing/loop knobs match the delta-timing harness: timing=True turns
the x inputs into device-initialized Internal DRAM; loop>1 wraps the body
in a hardware For_i for (wall(L2)-wall(L1))/(L2-L1) timing.
"""

from contextlib import ExitStack, nullcontext

import numpy as np

B = 2
S = 4096
D = 512
H = 8
DK = 64
P = 128
N_CORES = 8
FC = D // P  # 4 d_in chunks
NKT = S // P  # 32 kv tiles
NCH = S // 512  # 8 row chunks
QC = S // 512  # 8 query chunks
INV_SCALE = 0.125  # 1/sqrt(DK)

_CACHE = {}


def _build_nc(timing: bool = False, loop: int = 1):
    import concourse.mybir as mybir
    import concourse.tile as tile
    from concourse import bacc

    f32 = mybir.dt.float32
    f32r = mybir.dt.float32r
    bf16 = mybir.dt.bfloat16
    EXP = mybir.ActivationFunctionType.Exp

    nc = bacc.Bacc(
        "TRN2",
        target_bir_lowering=False,
        debug=False,
        enable_asserts=False,
        num_devices=N_CORES,
    )

    def din(name, shape, dt):
        kind = "Internal" if timing and name in ("xq", "xk", "xv") else "ExternalInput"
        return nc.dram_tensor(name, shape, dt, kind=kind).ap()

    xq = din("xq", [S, D], bf16)
    xk = din("xk", [S, D], bf16)
    xv = din("xv", [S, D], bf16)
    wq = din("wq", [P, D], bf16)
    wk = din("wk", [P, D], bf16)
    wv = din("wv", [P, D], bf16)
    wo = din("wo", [D, P], bf16)
    bq = din("bq", [1, P], f32)
    bk = din("bk", [1, P], f32)
    bv = din("bv", [1, P], f32)
    out = nc.dram_tensor("out", [S, D], f32, kind="ExternalOutput").ap()

    with tile.TileContext(nc) as tc:
        if timing:
            with tc.tile_pool(name="init", bufs=1) as initp:
                fill = initp.tile([P, D], bf16, name="fill")
                nc.vector.memset(fill, 0.01)
                for t_ap in (xq, xk, xv):
                    for rt in range(S // P):
                        nc.sync.dma_start(t_ap[rt * P : (rt + 1) * P, :], fill)

        with tc.For_i(0, loop, 1) if loop > 1 else nullcontext():
            st = ExitStack()
            consts = st.enter_context(tc.tile_pool(name="consts", bufs=1))

            # ---- tiles for constants (instructions emitted in the lead-in
            # below, ordered by what gates the first exp) ----
            ones = consts.tile([1, 512], f32r, name="ones")
            bqc = consts.tile([P, 1], f32, name="bqc")
            bkc = consts.tile([P, 1], f32, name="bkc")
            bvr = consts.tile([1, P], f32r, name="bvr")
            stg = st.enter_context(tc.tile_pool(name="stg", bufs=2))
            wTq = consts.tile([P, FC, P], bf16, name="wTq")
            wTk = consts.tile([P, FC, P], bf16, name="wTk")
            wTv = consts.tile([P, FC, P], bf16, name="wTv")
            wTo = consts.tile([P, FC * P], bf16, name="wTo")
            bvb = consts.tile([P, P], f32, name="bvb")

            # ---- PSUM pools: 4 (scores) + 2 (acc) + 2 (work) = 8 banks ----
            spsum = st.enter_context(tc.tile_pool(name="spsum", bufs=2, space="PSUM"))
            attacc = st.enter_context(tc.tile_pool(name="attacc", bufs=1, space="PSUM"))
            work_ps = st.enter_context(tc.tile_pool(name="wps", bufs=2, space="PSUM"))

            # ---- persistent projections ----
            qt_pool = st.enter_context(tc.tile_pool(name="QT", bufs=1))
            kt_pool = st.enter_context(tc.tile_pool(name="KT", bufs=1))
            vp_pool = st.enter_context(tc.tile_pool(name="Vp", bufs=1))
            QT = qt_pool.tile([P, S], bf16, name="QT")
            KT = kt_pool.tile([P, S], bf16, name="KT")
            Vp = vp_pool.tile([P, NKT, 2, DK + 1], bf16, name="Vp")

            # full transposed-x tiles; chunk slices written by single XBAR
            # transpose calls, read by per-chunk projections (range-tracked)
            xqt_pool = st.enter_context(tc.tile_pool(name="xqt", bufs=1))
            xkt_pool = st.enter_context(tc.tile_pool(name="xkt", bufs=1))
            xvt_pool = st.enter_context(tc.tile_pool(name="xvt", bufs=1))
            xqT = xqt_pool.tile([P, FC, S], bf16, name="xqT")
            xkT = xkt_pool.tile([P, FC, S], bf16, name="xkT")
            xvT = xvt_pool.tile([P, FC, S], bf16, name="xvT")

            def k_tr(c):
                # [512,512] DRAM chunk -> [128, 4, 512] transposed slice
                nc.sync.dma_start_transpose(
                    xkT[:, :, c * 512 : (c + 1) * 512],
                    xk[c * 512 : (c + 1) * 512, :],
                )

            def q_tr(c):
                nc.sync.dma_start_transpose(
                    xqT[:, :, c * 512 : (c + 1) * 512],
                    xq[c * 512 : (c + 1) * 512, :],
                )

            def v_tr(c):
                nc.sync.dma_start_transpose(
                    xvT[:, :, c * 512 : (c + 1) * 512],
                    xv[c * 512 : (c + 1) * 512, :],
                )

            def k_proj(c):
                ps = work_ps.tile([P, 512], f32, tag="w", name=f"kps{c}")
                for dic in range(FC):
                    nc.tensor.matmul(
                        ps,
                        lhsT=wTk[:, dic, :],
                        rhs=xkT[:, dic, c * 512 : (c + 1) * 512],
                        start=(dic == 0),
                        stop=(dic == FC - 1),
                    )
                nc.vector.tensor_scalar_add(
                    KT[:, c * 512 : (c + 1) * 512], in0=ps, scalar1=bkc
                )

            def q_proj(c):
                ps = work_ps.tile([P, 512], f32, tag="w", name=f"qps{c}")
                for dic in range(FC):
                    nc.tensor.matmul(
                        ps,
                        lhsT=wTq[:, dic, :],
                        rhs=xqT[:, dic, c * 512 : (c + 1) * 512],
                        start=(dic == 0),
                        stop=(dic == FC - 1),
                    )
                nc.vector.tensor_scalar_add(
                    QT[:, c * 512 : (c + 1) * 512], in0=ps, scalar1=bqc
                )

            def v_proj(c, rt2):
                for rt in (2 * rt2, 2 * rt2 + 1):
                    kt = c * 4 + rt
                    ps = work_ps.tile([P, 512], f32, tag="w", name=f"vps{c}_{rt}")
                    for dic in range(FC):
                        nc.tensor.matmul(
                            ps[:, 0:P],
                            lhsT=xvT[:, dic, c * 512 + rt * P : c * 512 + (rt + 1) * P],
                            rhs=wTv[:, dic, :],
                            start=(dic == 0),
                            stop=(dic == FC - 1),
                        )
                    nc.vector.tensor_add(
                        Vp[:, kt, :, 0:DK],
                        ps[:, 0:P].rearrange("p (h d) -> p h d", h=2),
                        bvb.rearrange("p (h d) -> p h d", h=2),
                    )
                if rt2 == 1:
                    nc.vector.memset(Vp[:, c * 4 : (c + 1) * 4, :, DK : DK + 1], 1.0)

            # ---- lead-in, ordered by what gates the first exp ----
            # exp table load first (pure ACT), then the score-side chain
            # (wTk/wTq, x transposes, projections), then the AV side (bvb,
            # wTv, V0), then wTo (needed only at the end of qc0).
            warm_src = consts.tile([1, 8], f32, name="warm_src")
            warm = consts.tile([1, 8], bf16, name="warm")
            nc.vector.memset(warm_src, 0.0)
            nc.scalar.activation(warm, warm_src, func=EXP)

            ones_stg = stg.tile([1, 512], f32, tag="s", name="ones_stg")
            nc.vector.memset(ones_stg, 1.0)
            nc.vector.tensor_copy(ones, ones_stg)
            nc.sync.dma_start_transpose(wTk, wk)
            nc.sync.dma_start_transpose(wTq, wq)
            k_tr(0)
            q_tr(0)
            # bias columns [P,1] via PE broadcast: a strided [1,P]->[P,1]
            # DMA is 128 descriptors and would park the SP queue; a
            # contiguous load + ones-matmul is ~free
            ones_bf = stg.tile([1, 8], bf16, tag="ob", name="ones_bf")
            nc.vector.memset(ones_bf, 1.0)
            for bap, bcol, nm in ((bk, bkc, "bk"), (bq, bqc, "bq")):
                bstg = stg.tile([1, P], f32, tag="s", name=f"stg_{nm}")
                nc.sync.dma_start(bstg, bap)
                brr = stg.tile([1, P], bf16, tag="sr", name=f"r_{nm}")
                nc.vector.tensor_copy(brr, bstg)
                psb = work_ps.tile([P, 512], f32, tag="w", name=f"psb_{nm}")
                nc.tensor.matmul(
                    psb[:, 0:8], lhsT=brr, rhs=ones_bf, start=True, stop=True
                )
                nc.vector.tensor_copy(bcol, psb[:, 0:1])
            k_proj(0)
            q_proj(0)

            # get K1/V1 into the serial SP queue as early as possible: the
            # whole qc0 is paced by this transpose train (~2.5us/chunk)
            nc.sync.dma_start_transpose(wTv, wv)
            v_tr(0)
            k_tr(1)
            v_tr(1)
            bvs = stg.tile([1, P], f32, tag="s", name="bvs")
            nc.sync.dma_start(bvs, bv)
            nc.vector.tensor_copy(bvr, bvs)
            # bvb[s, dv] = bv[dv] broadcast over partitions (ones matmul)
            ps_bvb = work_ps.tile([P, 512], f32, tag="w", name="psbvb")
            nc.tensor.matmul(
                ps_bvb[:, 0:P], lhsT=ones[:, 0:P], rhs=bvr, start=True, stop=True
            )
            nc.vector.tensor_copy(bvb, ps_bvb[:, 0:P])
            v_proj(0, 0)
            v_proj(0, 1)

            # remaining transposes upfront, ordered by need time (K_c/V_c at
            # qc0 kt=4c; wTo at qc0's epilogue; Q_c at the qc=c boundary)
            for c in range(2, NCH):
                k_tr(c)
                v_tr(c)
                if c == 2:
                    # wo [512,128] -> [128 dh, 512 do] in one call
                    nc.sync.dma_start_transpose(wTo, wo)
                if c == 4:
                    q_tr(1)
            for c in range(2, NCH):
                q_tr(c)

            # K/V projections are paced inside qc0 (3 steps per chunk);
            # Q chunk c is projected one query-chunk early.
            queue = []
            for c in range(1, NCH):
                queue.extend(
                    [
                        lambda c=c: k_proj(c),
                        lambda c=c: v_proj(c, 0),
                        lambda c=c: v_proj(c, 1),
                    ]
                )
            qqueue = [lambda c=c: q_proj(c) for c in range(1, NCH)]
            pos = 0

            # ---- attention + normalize + partial out projection ----
            ex_pool = st.enter_context(tc.tile_pool(name="ex", bufs=4))
            attT_pool = st.enter_context(tc.tile_pool(name="attT", bufs=2))
            small = st.enter_context(tc.tile_pool(name="small", bufs=2))
            outbuf = st.enter_context(tc.tile_pool(name="outbuf", bufs=2))

            defer_parts = []
            for qc in range(QC):
                qs = slice(qc * 512, (qc + 1) * 512)
                acc = [
                    attacc.tile([DK + 1, 512], f32, tag=f"acc{i}", name=f"acc{qc}_{i}")
                    for i in range(2)
                ]
                pend = None
                for kt in range(NKT):
                    if pos < len(queue):
                        # emit chunk c's projections at kt=4c-1, one unit
                        # before their scores consume them: any earlier and
                        # the in-order PE parks on the pending SP transposes
                        target = (
                            min(len(queue), 3 * ((kt + 1) // 4))
                            if qc == 0
                            else len(queue)
                        )
                        while pos < target:
                            queue[pos]()
                            pos += 1
                    if qc < QC - 1 and kt == 24:
                        # project Q chunk qc+1 late in this query-chunk: its
                        # transpose sits behind K/V in the SP DMA queue, and
                        # an earlier emission would park PE (in-order) on it
                        qqueue[qc]()
                    ks = slice(kt * P, (kt + 1) * P)
                    sc = spsum.tile([P, 2, 512], f32, tag="sc", name=f"sc{qc}_{kt}")
                    for i in range(2):
                        nc.tensor.matmul(
                            sc[:, i, :],
                            lhsT=KT[i * DK : (i + 1) * DK, ks],
                            rhs=QT[i * DK : (i + 1) * DK, qs],
                            start=True,
                            stop=True,
                        )
                    ex = ex_pool.tile([P, 2, 512], bf16, tag="ex", name=f"ex{qc}_{kt}")
                    nc.scalar.activation(ex, sc, func=EXP, scale=INV_SCALE)
                    # previous qc's epilogue part A (pure DVE: copies acc out
                    # of PSUM) must be emitted BEFORE this qc's first AV write
                    # reuses the acc banks
                    if kt == 1 and defer_parts:
                        defer_parts[0]()
                    # software pipeline: AV of unit kt-1 is emitted AFTER the
                    # scores of unit kt. PE is in-order, so emitting AV(kt)
                    # here would make its wait-for-exp(kt) block the next
                    # scores and serialize the whole loop at exp+av+sc.
                    if pend is not None:
                        pex, pkt = pend
                        for i in range(2):
                            nc.tensor.matmul(
                                acc[i],
                                lhsT=Vp[:, pkt, i, :],
                                rhs=pex[:, i, :],
                                start=(pkt == 0),
                                stop=False,
                            )
                    pend = (ex, kt)
                    # previous qc's epilogue parts B..D: small PE batches
                    # spread one per unit so they hide in the PE slack
                    if 2 <= kt <= 4 and defer_parts:
                        defer_parts[kt - 1]()
                        if kt == 4:
                            defer_parts = []
                # flush the pipelined last AV (kt = NKT-1, closes the group)
                pex, pkt = pend
                for i in range(2):
                    nc.tensor.matmul(
                        acc[i],
                        lhsT=Vp[:, pkt, i, :],
                        rhs=pex[:, i, :],
                        start=False,
                        stop=True,
                    )

                def make_finish(qc=qc, acc=acc):
                    state = {}

                    def part_a():
                        # DVE only: move acc out of PSUM, reciprocals
                        for i in range(2):
                            acc_sb = small.tile(
                                [DK + 1, 512], f32, tag="acc_sb", name=f"asb{qc}_{i}"
                            )
                            nc.vector.tensor_copy(acc_sb, acc[i])
                            rc = small.tile(
                                [1, 512], f32r, tag="rc", name=f"rc{qc}_{i}"
                            )
                            with nc.allow_low_precision(reason="f32r recip denom"):
                                nc.vector.reciprocal(rc, acc_sb[DK : DK + 1, :])
                            state[i] = (acc_sb, rc)

                    def part_b():
                        # broadcast 1/denom over 64 partitions, normalize
                        attT = attT_pool.tile(
                            [P, 512], bf16, tag="attT", name=f"attT{qc}"
                        )
                        state["attT"] = attT
                        for i in range(2):
                            acc_sb, rc = state[i]
                            ps_rb = work_ps.tile(
                                [P, 512], f32, tag="w", name=f"rb{qc}_{i}"
                            )
                            nc.tensor.matmul(
                                ps_rb[0:DK, :],
                                lhsT=ones[:, 0:DK],
                                rhs=rc,
                                start=True,
                                stop=True,
                            )
                            rb = small.tile(
                                [DK, 512], f32, tag="rb", name=f"rb_sb{qc}_{i}"
                            )
                            nc.vector.tensor_copy(rb, ps_rb[0:DK, :])
                            nc.vector.tensor_mul(
                                attT[i * DK : (i + 1) * DK, :],
                                acc_sb[0:DK, :],
                                rb,
                            )

                    def part_c(qt2):
                        # partial out projection: po[q, do] = attT.T @ wTo
                        attT = state["attT"]
                        for qt in (2 * qt2, 2 * qt2 + 1):
                            po = work_ps.tile(
                                [P, 512], f32, tag="w", name=f"po{qc}_{qt}"
                            )
                            nc.tensor.matmul(
                                po,
                                lhsT=attT[:, qt * P : (qt + 1) * P],
                                rhs=wTo,
                                start=True,
                                stop=True,
                            )
                            ot = outbuf.tile(
                                [P, 512], f32, tag="ot", name=f"ot{qc}_{qt}"
                            )
                            nc.vector.tensor_copy(ot, po)
                            nc.sync.dma_start(
                                out[qc * 512 + qt * P : qc * 512 + (qt + 1) * P, :],
                                ot,
                            )

                    return [part_a, part_b, lambda: part_c(0), lambda: part_c(1)]

                defer_parts = make_finish()
            for f in defer_parts:
                f()
            st.close()

    nc.compile()
    return nc


def get_nc(timing: bool = False, loop: int = 1):
    key = f"v2{'t' if timing else ''}l{loop}"
    if key not in _CACHE:
        _CACHE[key] = _build_nc(timing, loop)
    return _CACHE[key]


def make_in_maps(query, key, value, w_q, b_q, w_k, b_k, w_v, b_v, w_o, b_o):
    import ml_dtypes

    bf = ml_dtypes.bfloat16
    query = np.asarray(query, np.float32)
    key = np.asarray(key, np.float32)
    value = np.asarray(value, np.float32)
    w_q = np.asarray(w_q, np.float32)
    w_k = np.asarray(w_k, np.float32)
    w_v = np.asarray(w_v, np.float32)
    w_o = np.asarray(w_o, np.float32)
    b_q = np.asarray(b_q, np.float32).reshape(-1)
    b_k = np.asarray(b_k, np.float32).reshape(-1)
    b_v = np.asarray(b_v, np.float32).reshape(-1)

    xb = {}
    for b in range(B):
        xb[b] = (
            np.ascontiguousarray(query[b]).astype(bf),
            np.ascontiguousarray(key[b]).astype(bf),
            np.ascontiguousarray(value[b]).astype(bf),
        )
    in_maps = []
    for c in range(N_CORES):
        b, hp = c // 4, c % 4
        sl = slice(hp * P, (hp + 1) * P)
        in_maps.append(
            {
                "xq": xb[b][0],
                "xk": xb[b][1],
                "xv": xb[b][2],
                "wq": np.ascontiguousarray(w_q[sl, :]).astype(bf),
                "wk": np.ascontiguousarray(w_k[sl, :]).astype(bf),
                "wv": np.ascontiguousarray(w_v[sl, :]).astype(bf),
                "wo": np.ascontiguousarray(w_o[:, sl]).astype(bf),
                "bq": np.ascontiguousarray(b_q[sl]).reshape(1, P),
                "bk": np.ascontiguousarray(b_k[sl]).reshape(1, P),
                "bv": np.ascontiguousarray(b_v[sl]).reshape(1, P),
            }
        )
    return in_maps


def assemble(outs, b_o):
    """Sum the 4 per-head-pair partials per batch and add b_o."""
    bo = np.asarray(b_o, np.float32).reshape(1, D)
    full = np.empty((B, S, D), np.float32)
    for b in range(B):
        acc = outs[4 * b].astype(np.float32).copy()
        for hp in range(1, 4):
            acc += outs[4 * b + hp]
        full[b] = acc + bo
    return full


def kernel(query, key, value, w_q, b_q, w_k, b_k, w_v, b_v, w_o, b_o):
    from concourse import bass_utils

    in_maps = make_in_maps(
        query, key, value, w_q, b_q, w_k, b_k, w_v, b_v, w_o, b_o
    )
    nc = get_nc()
    for attempt in range(2):
        res = bass_utils.run_bass_kernel_spmd(
            nc, in_maps, core_ids=list(range(N_CORES))
        )
        outs = [res.results[c]["out"] for c in range(N_CORES)]
        full = assemble(outs, b_o)
        if np.isfinite(full).all():
            break
        # rare fresh-process first-dispatch flake: retry once
    return full


if __name__ == "__main__":
    nc = get_nc()
    print("built ok")


# revision 38
# speedup vs baseline: 1.2084x; 1.2084x over previous
"""Multi-head attention Trainium2 Bass kernel, 8-way SPMD, v2.

Problem: nn_MultiHeadAttention (B=2, S=4096, D=512, H=8, Dk=64), fp32 I/O.

Sharding v2: (batch, head-pair). Core c = (batch c//4, head-pair c%4).
Each core computes its two heads' Q/K/V projections over the FULL 4096-row
sequence of its batch (no projection duplication), runs attention for those
heads, and produces a PARTIAL output projection (its 128 attended dims
through the matching w_o rows). The host sums the 4 partials per batch and
adds b_o.

Engine budget per core (the kernel is ACT-bound; HW-measured rates):
  ACT  : 256 exp instructions over [128,1024] PSUM scores, 1015 ns each
         (HW-measured stream rate) ~260 us  <- hard floor
  PE   : scores (head-pair row-group concurrency, 255 ns/pair measured)
         + AV (ones-column denom, 552 ns/pair) + projections  ~220 us
  DVE  : bias adds, normalize, psum->sbuf out copies           ~82 us
  DMA  : bf16 x/w loaded transposed via XBAR dma_start_transpose
         ([512,512] chunk call = 2.45 us measured; all on the SP hwdge
         queue - ACT-queue DMA dispatch races on this runtime), ~60 us

Scheduling rules this kernel is built around (all HW-verified here):
  - PE executes strictly in order: any matmul that waits on an ACT/DVE
    result blocks every later matmul. AV(kt) is therefore emitted after
    scores(kt+1), and the per-qc epilogue (normalize + out proj) is
    deferred into the next qc's kt loop in small PE batches.
  - Production (K/V/Q projections) is paced one chunk ahead of its
    consumption so its PSUM-wait never parks the PE queue.
  - All XBAR transposes are emitted upfront, ordered by need time; the
    SP DMA queue drains them (~2.5 us each) under the attention stream.

Key layout facts:
  - dma_start_transpose([128,512] DRAM bf16) -> [128,4,128] SBUF gives
    xT[p, dic, j] = x[j, dic*128+p] directly (verified on HW).
  - KT/QT [128, 4096] bf16 hold the head pair stacked (head0 dims on
    partitions 0..63, head1 on 64..127), so the two score matmuls use PE
    row-groups 0/64 and run concurrently (tile_position auto-derived).
  - V' = [V_h | 1] per head (65 cols): the AV matmul accumulates the
    softmax denominator in PSUM row 64 for free.

repeat/timing/loop knobs match the delta-timing harness: timing=True turns
the x inputs into device-initialized Internal DRAM; loop>1 wraps the body
in a hardware For_i for (wall(L2)-wall(L1))/(L2-L1) timing.
"""

from contextlib import ExitStack, nullcontext

import numpy as np

B = 2
S = 4096
D = 512
H = 8
DK = 64
P = 128
N_CORES = 8
FC = D // P  # 4 d_in chunks
NKT = S // P  # 32 kv tiles
NCH = S // 512  # 8 row chunks
QC = S // 512  # 8 query chunks
INV_SCALE = 0.125  # 1/sqrt(DK)

_CACHE = {}


def _build_nc(timing: bool = False, loop: int = 1):
    import concourse.mybir as mybir
    import concourse.tile as tile
    from concourse import bacc

    f32 = mybir.dt.float32
    f32r = mybir.dt.float32r
    bf16 = mybir.dt.bfloat16
    EXP = mybir.ActivationFunctionType.Exp

    nc = bacc.Bacc(
        "TRN2",
        target_bir_lowering=False,
        debug=False,
        enable_asserts=False,
        num_devices=N_CORES,
    )

    def din(name, shape, dt):
        kind = "Internal" if timing and name in ("xq", "xk", "xv") else "ExternalInput"
        return nc.dram_tensor(name, shape, dt, kind=kind).ap()

    xq = din("xq", [S, D], bf16)
    xk = din("xk", [S, D], bf16)
    xv = din("xv", [S, D], bf16)
    wq = din("wq", [P, D], bf16)
    wk = din("wk", [P, D], bf16)
    wv = din("wv", [P, D], bf16)
    wo = din("wo", [D, P], bf16)
    bq = din("bq", [1, P], f32)
    bk = din("bk", [1, P], f32)
    bv = din("bv", [1, P], f32)
    out = nc.dram_tensor("out", [S, D], f32, kind="ExternalOutput").ap()

    with tile.TileContext(nc) as tc:
        if timing:
            with tc.tile_pool(name="init", bufs=1) as initp:
                fill = initp.tile([P, D], bf16, name="fill")
                nc.vector.memset(fill, 0.01)
                for t_ap in (xq, xk, xv):
                    for rt in range(S // P):
                        nc.sync.dma_start(t_ap[rt * P : (rt + 1) * P, :], fill)

        with tc.For_i(0, loop, 1) if loop > 1 else nullcontext():
            st = ExitStack()
            consts = st.enter_context(tc.tile_pool(name="consts", bufs=1))

            # ---- tiles for constants (instructions emitted in the lead-in
            # below, ordered by what gates the first exp) ----
            ones = consts.tile([1, 512], f32r, name="ones")
            bqc = consts.tile([P, 1], f32, name="bqc")
            bkc = consts.tile([P, 1], f32, name="bkc")
            bvr = consts.tile([1, P], f32r, name="bvr")
            stg = st.enter_context(tc.tile_pool(name="stg", bufs=2))
            wTq = consts.tile([P, FC, P], bf16, name="wTq")
            wTk = consts.tile([P, FC, P], bf16, name="wTk")
            wTv = consts.tile([P, FC, P], bf16, name="wTv")
            wTo = consts.tile([P, FC * P], bf16, name="wTo")
            bvb = consts.tile([P, P], f32, name="bvb")

            # ---- PSUM pools: 4 (scores) + 2 (acc) + 2 (work) = 8 banks ----
            spsum = st.enter_context(tc.tile_pool(name="spsum", bufs=2, space="PSUM"))
            attacc = st.enter_context(tc.tile_pool(name="attacc", bufs=1, space="PSUM"))
            work_ps = st.enter_context(tc.tile_pool(name="wps", bufs=2, space="PSUM"))

            # ---- persistent projections ----
            qt_pool = st.enter_context(tc.tile_pool(name="QT", bufs=1))
            kt_pool = st.enter_context(tc.tile_pool(name="KT", bufs=1))
            vp_pool = st.enter_context(tc.tile_pool(name="Vp", bufs=1))
            QT = qt_pool.tile([P, S], bf16, name="QT")
            KT = kt_pool.tile([P, S], bf16, name="KT")
            Vp = vp_pool.tile([P, NKT, 2, DK + 1], bf16, name="Vp")

            # full transposed-x tiles; chunk slices written by single XBAR
            # transpose calls, read by per-chunk projections (range-tracked)
            xqt_pool = st.enter_context(tc.tile_pool(name="xqt", bufs=1))
            xkt_pool = st.enter_context(tc.tile_pool(name="xkt", bufs=1))
            xvt_pool = st.enter_context(tc.tile_pool(name="xvt", bufs=1))
            xqT = xqt_pool.tile([P, FC, S], bf16, name="xqT")
            xkT = xkt_pool.tile([P, FC, S], bf16, name="xkT")
            xvT = xvt_pool.tile([P, FC, S], bf16, name="xvT")

            def k_tr(c):
                # [512,512] DRAM chunk -> [128, 4, 512] transposed slice
                nc.sync.dma_start_transpose(
                    xkT[:, :, c * 512 : (c + 1) * 512],
                    xk[c * 512 : (c + 1) * 512, :],
                )

            def q_tr(c):
                nc.sync.dma_start_transpose(
                    xqT[:, :, c * 512 : (c + 1) * 512],
                    xq[c * 512 : (c + 1) * 512, :],
                )

            def v_tr(c):
                nc.sync.dma_start_transpose(
                    xvT[:, :, c * 512 : (c + 1) * 512],
                    xv[c * 512 : (c + 1) * 512, :],
                )

            def k_proj(c):
                ps = work_ps.tile([P, 512], f32, tag="w", name=f"kps{c}")
                for dic in range(FC):
                    nc.tensor.matmul(
                        ps,
                        lhsT=wTk[:, dic, :],
                        rhs=xkT[:, dic, c * 512 : (c + 1) * 512],
                        start=(dic == 0),
                        stop=(dic == FC - 1),
                    )
                nc.vector.tensor_scalar_add(
                    KT[:, c * 512 : (c + 1) * 512], in0=ps, scalar1=bkc
                )

            def q_proj(c):
                ps = work_ps.tile([P, 512], f32, tag="w", name=f"qps{c}")
                for dic in range(FC):
                    nc.tensor.matmul(
                        ps,
                        lhsT=wTq[:, dic, :],
                        rhs=xqT[:, dic, c * 512 : (c + 1) * 512],
                        start=(dic == 0),
                        stop=(dic == FC - 1),
                    )
                nc.vector.tensor_scalar_add(
                    QT[:, c * 512 : (c + 1) * 512], in0=ps, scalar1=bqc
                )

            def v_proj(c, rt2):
                for rt in (2 * rt2, 2 * rt2 + 1):
                    kt = c * 4 + rt
                    ps = work_ps.tile([P, 512], f32, tag="w", name=f"vps{c}_{rt}")
                    for dic in range(FC):
                        nc.tensor.matmul(
                            ps[:, 0:P],
                            lhsT=xvT[:, dic, c * 512 + rt * P : c * 512 + (rt + 1) * P],
                            rhs=wTv[:, dic, :],
                            start=(dic == 0),
                            stop=(dic == FC - 1),
                        )
                    nc.vector.tensor_add(
                        Vp[:, kt, :, 0:DK],
                        ps[:, 0:P].rearrange("p (h d) -> p h d", h=2),
                        bvb.rearrange("p (h d) -> p h d", h=2),
                    )
                if rt2 == 1:
                    nc.vector.memset(Vp[:, c * 4 : (c + 1) * 4, :, DK : DK + 1], 1.0)

            # ---- lead-in, ordered by what gates the first exp ----
            # exp table load first (pure ACT), then the score-side chain
            # (wTk/wTq, x transposes, projections), then the AV side (bvb,
            # wTv, V0), then wTo (needed only at the end of qc0).
            warm_src = consts.tile([1, 8], f32, name="warm_src")
            warm = consts.tile([1, 8], bf16, name="warm")
            nc.vector.memset(warm_src, 0.0)
            nc.scalar.activation(warm, warm_src, func=EXP)

            ones_stg = stg.tile([1, 512], f32, tag="s", name="ones_stg")
            nc.vector.memset(ones_stg, 1.0)
            nc.vector.tensor_copy(ones, ones_stg)
            nc.sync.dma_start_transpose(wTk, wk)
            nc.sync.dma_start_transpose(wTq, wq)
            k_tr(0)
            q_tr(0)
            # bias columns [P,1] via PE broadcast: a strided [1,P]->[P,1]
            # DMA is 128 descriptors and would park the SP queue; a
            # contiguous load + ones-matmul is ~free
            ones_bf = stg.tile([1, 8], bf16, tag="ob", name="ones_bf")
            nc.vector.memset(ones_bf, 1.0)
            for bap, bcol, nm in ((bk, bkc, "bk"), (bq, bqc, "bq")):
                bstg = stg.tile([1, P], f32, tag="s", name=f"stg_{nm}")
                nc.sync.dma_start(bstg, bap)
                brr = stg.tile([1, P], bf16, tag="sr", name=f"r_{nm}")
                nc.vector.tensor_copy(brr, bstg)
                psb = work_ps.tile([P, 512], f32, tag="w", name=f"psb_{nm}")
                nc.tensor.matmul(
                    psb[:, 0:8], lhsT=brr, rhs=ones_bf, start=True, stop=True
                )
                nc.vector.tensor_copy(bcol, psb[:, 0:1])
            k_proj(0)
            q_proj(0)

            nc.sync.dma_start_transpose(wTv, wv)
            bvs = stg.tile([1, P], f32, tag="s", name="bvs")
            nc.sync.dma_start(bvs, bv)
            nc.vector.tensor_copy(bvr, bvs)
            # bvb[s, dv] = bv[dv] broadcast over partitions (ones matmul)
            ps_bvb = work_ps.tile([P, 512], f32, tag="w", name="psbvb")
            nc.tensor.matmul(
                ps_bvb[:, 0:P], lhsT=ones[:, 0:P], rhs=bvr, start=True, stop=True
            )
            nc.vector.tensor_copy(bvb, ps_bvb[:, 0:P])
            v_tr(0)
            v_proj(0, 0)
            v_proj(0, 1)

            # all remaining transposes upfront, ordered by need time; the SP
            # DMA queue works through them (~2.5us each) while attention runs.
            # K_c/V_c are consumed at qc0 kt=4c; Q_c at the qc=c boundary.
            # wTo (epilogue-only) and q_tr(1) ride mid-train after K3/V3.
            for c in range(1, 6):
                k_tr(c)
                v_tr(c)
                if c == 3:
                    # wo [512,128] -> [128 dh, 512 do] in one call
                    nc.sync.dma_start_transpose(wTo, wo)
            q_tr(1)
            for c in range(6, NCH):
                k_tr(c)
                v_tr(c)
            for c in range(2, NCH):
                q_tr(c)

            # K/V projections are paced inside qc0 (3 steps per chunk);
            # Q chunk c is projected one query-chunk early.
            queue = []
            for c in range(1, NCH):
                queue.extend(
                    [
                        lambda c=c: k_proj(c),
                        lambda c=c: v_proj(c, 0),
                        lambda c=c: v_proj(c, 1),
                    ]
                )
            qqueue = [lambda c=c: q_proj(c) for c in range(1, NCH)]
            pos = 0

            # ---- attention + normalize + partial out projection ----
            ex_pool = st.enter_context(tc.tile_pool(name="ex", bufs=4))
            attT_pool = st.enter_context(tc.tile_pool(name="attT", bufs=2))
            small = st.enter_context(tc.tile_pool(name="small", bufs=3))
            outbuf = st.enter_context(tc.tile_pool(name="outbuf", bufs=3))

            defer_parts = []
            for qc in range(QC):
                qs = slice(qc * 512, (qc + 1) * 512)
                acc = [
                    attacc.tile([DK + 1, 512], f32, tag=f"acc{i}", name=f"acc{qc}_{i}")
                    for i in range(2)
                ]
                pend = None
                for kt in range(NKT):
                    if pos < len(queue):
                        # keep K/V projections ONE chunk ahead of consumption:
                        # emitting earlier parks in-order PE on the pending
                        # SP-queue transposes
                        target = min(len(queue), 3 * (kt // 4 + 1)) if qc == 0 else len(queue)
                        while pos < target:
                            queue[pos]()
                            pos += 1
                    if qc < QC - 1 and kt == 24:
                        # project Q chunk qc+1 late in this query-chunk: its
                        # transpose sits behind K/V in the SP DMA queue, and
                        # an earlier emission would park PE (in-order) on it
                        qqueue[qc]()
                    ks = slice(kt * P, (kt + 1) * P)
                    sc = spsum.tile([P, 2, 512], f32, tag="sc", name=f"sc{qc}_{kt}")
                    for i in range(2):
                        nc.tensor.matmul(
                            sc[:, i, :],
                            lhsT=KT[i * DK : (i + 1) * DK, ks],
                            rhs=QT[i * DK : (i + 1) * DK, qs],
                            start=True,
                            stop=True,
                        )
                    ex = ex_pool.tile([P, 2, 512], bf16, tag="ex", name=f"ex{qc}_{kt}")
                    nc.scalar.activation(ex, sc, func=EXP, scale=INV_SCALE)
                    # previous qc's epilogue part A (pure DVE: copies acc out
                    # of PSUM) must be emitted BEFORE this qc's first AV write
                    # reuses the acc banks
                    if kt == 1 and defer_parts:
                        defer_parts[0]()
                    # software pipeline: AV of unit kt-1 is emitted AFTER the
                    # scores of unit kt. PE is in-order, so emitting AV(kt)
                    # here would make its wait-for-exp(kt) block the next
                    # scores and serialize the whole loop at exp+av+sc.
                    if pend is not None:
                        pex, pkt = pend
                        for i in range(2):
                            nc.tensor.matmul(
                                acc[i],
                                lhsT=Vp[:, pkt, i, :],
                                rhs=pex[:, i, :],
                                start=(pkt == 0),
                                stop=False,
                            )
                    pend = (ex, kt)
                    # previous qc's epilogue parts B..D: small PE batches
                    # spread one per unit so they hide in the PE slack
                    if 2 <= kt <= 4 and defer_parts:
                        defer_parts[kt - 1]()
                        if kt == 4:
                            defer_parts = []
                # flush the pipelined last AV (kt = NKT-1, closes the group)
                pex, pkt = pend
                for i in range(2):
                    nc.tensor.matmul(
                        acc[i],
                        lhsT=Vp[:, pkt, i, :],
                        rhs=pex[:, i, :],
                        start=False,
                        stop=True,
                    )

                def make_finish(qc=qc, acc=acc):
                    state = {}

                    def part_a():
                        # DVE only: move acc out of PSUM, reciprocals
                        for i in range(2):
                            acc_sb = small.tile(
                                [DK + 1, 512], f32, tag="acc_sb", name=f"asb{qc}_{i}"
                            )
                            nc.vector.tensor_copy(acc_sb, acc[i])
                            rc = small.tile(
                                [1, 512], f32r, tag="rc", name=f"rc{qc}_{i}"
                            )
                            with nc.allow_low_precision(reason="f32r recip denom"):
                                nc.vector.reciprocal(rc, acc_sb[DK : DK + 1, :])
                            state[i] = (acc_sb, rc)

                    def part_b():
                        # broadcast 1/denom over 64 partitions, normalize
                        attT = attT_pool.tile(
                            [P, 512], bf16, tag="attT", name=f"attT{qc}"
                        )
                        state["attT"] = attT
                        for i in range(2):
                            acc_sb, rc = state[i]
                            ps_rb = work_ps.tile(
                                [P, 512], f32, tag="w", name=f"rb{qc}_{i}"
                            )
                            nc.tensor.matmul(
                                ps_rb[0:DK, :],
                                lhsT=ones[:, 0:DK],
                                rhs=rc,
                                start=True,
                                stop=True,
                            )
                            rb = small.tile(
                                [DK, 512], f32, tag="rb", name=f"rb_sb{qc}_{i}"
                            )
                            nc.vector.tensor_copy(rb, ps_rb[0:DK, :])
                            nc.vector.tensor_mul(
                                attT[i * DK : (i + 1) * DK, :],
                                acc_sb[0:DK, :],
                                rb,
                            )

                    def part_c(qt2):
                        # partial out projection: po[q, do] = attT.T @ wTo
                        attT = state["attT"]
                        for qt in (2 * qt2, 2 * qt2 + 1):
                            po = work_ps.tile(
                                [P, 512], f32, tag="w", name=f"po{qc}_{qt}"
                            )
                            nc.tensor.matmul(
                                po,
                                lhsT=attT[:, qt * P : (qt + 1) * P],
                                rhs=wTo,
                                start=True,
                                stop=True,
                            )
                            ot = outbuf.tile(
                                [P, 512], f32, tag="ot", name=f"ot{qc}_{qt}"
                            )
                            nc.vector.tensor_copy(ot, po)
                            nc.sync.dma_start(
                                out[qc * 512 + qt * P : qc * 512 + (qt + 1) * P, :],
                                ot,
                            )

                    return [part_a, part_b, lambda: part_c(0), lambda: part_c(1)]

                defer_parts = make_finish()
            for f in defer_parts:
                f()
            st.close()

    nc.compile()
    return nc


def get_nc(timing: bool = False, loop: int = 1):
    key = f"v2{'t' if timing else ''}l{loop}"
    if key not in _CACHE:
        _CACHE[key] = _build_nc(timing, loop)
    return _CACHE[key]


def make_in_maps(query, key, value, w_q, b_q, w_k, b_k, w_v, b_v, w_o, b_o):
    import ml_dtypes

    bf = ml_dtypes.bfloat16
    query = np.asarray(query, np.float32)
    key = np.asarray(key, np.float32)
    value = np.asarray(value, np.float32)
    w_q = np.asarray(w_q, np.float32)
    w_k = np.asarray(w_k, np.float32)
    w_v = np.asarray(w_v, np.float32)
    w_o = np.asarray(w_o, np.float32)
    b_q = np.asarray(b_q, np.float32).reshape(-1)
    b_k = np.asarray(b_k, np.float32).reshape(-1)
    b_v = np.asarray(b_v, np.float32).reshape(-1)

    xb = {}
    for b in range(B):
        xb[b] = (
            np.ascontiguousarray(query[b]).astype(bf),
            np.ascontiguousarray(key[b]).astype(bf),
            np.ascontiguousarray(value[b]).astype(bf),
        )
    in_maps = []
    for c in range(N_CORES):
        b, hp = c // 4, c % 4
        sl = slice(hp * P, (hp + 1) * P)
        in_maps.append(
            {
                "xq": xb[b][0],
                "xk": xb[b][1],
                "xv": xb[b][2],
                "wq": np.ascontiguousarray(w_q[sl, :]).astype(bf),
                "wk": np.ascontiguousarray(w_k[sl, :]).astype(bf),
                "wv": np.ascontiguousarray(w_v[sl, :]).astype(bf),
                "wo": np.ascontiguousarray(w_o[:, sl]).astype(bf),
                "bq": np.ascontiguousarray(b_q[sl]).reshape(1, P),
                "bk": np.ascontiguousarray(b_k[sl]).reshape(1, P),
                "bv": np.ascontiguousarray(b_v[sl]).reshape(1, P),
            }
        )
    return in_maps


def assemble(outs, b_o):
    """Sum the 4 per-head-pair partials per batch and add b_o."""
    bo = np.asarray(b_o, np.float32).reshape(1, D)
    full = np.empty((B, S, D), np.float32)
    for b in range(B):
        acc = outs[4 * b].astype(np.float32).copy()
        for hp in range(1, 4):
            acc += outs[4 * b + hp]
        full[b] = acc + bo
    return full


def kernel(query, key, value, w_q, b_q, w_k, b_k, w_v, b_v, w_o, b_o):
    from concourse import bass_utils

    in_maps = make_in_maps(
        query, key, value, w_q, b_q, w_k, b_k, w_v, b_v, w_o, b_o
    )
    nc = get_nc()
    for attempt in range(2):
        res = bass_utils.run_bass_kernel_spmd(
            nc, in_maps, core_ids=list(range(N_CORES))
        )
        outs = [res.results[c]["out"] for c in range(N_CORES)]
        full = assemble(outs, b_o)
        if np.isfinite(full).all():
            break
        # rare fresh-process first-dispatch flake: retry once
    return full


if __name__ == "__main__":
    nc = get_nc()
    print("built ok")
